# revision 23
# baseline (speedup 1.0000x reference)
"""Mixture-of-Experts (top-2 of 8) Trainium2 kernel, expert-parallel over 8 NeuronCores.

Strategy (per the expert-parallel sharding hint):
  Launch A (data-parallel gating): each core computes gating logits for T/8
    tokens (x_slice @ Wg on the PE in fp16, 1 cycle/row), then top-2
    selection + renormalized combine weights with vector/scalar ops.
    Output: dense [T, E] combine weights (zero for unselected experts).
  Host routing ("all-to-all dispatch"): from the device-computed combine
    weights, build per-expert token index lists, gather+transpose+bf16-cast
    the routed tokens for each expert, pad to a common capacity C.
  Launch B (expert-parallel FFN): core e holds expert e's weights. Computes
    h^T = gelu(W1^T x^T + b1), y^T = (W2^T h^T + b2) * w on the PE in bf16
    with fp32 accumulation; biases added exactly in fp32 on the scalar
    engine; combine weight applied on the vector engine.
  Host unshard: scatter-add the 8 weighted partial outputs into [T, D].

Perf notes (measured on HW):
  - Both launches begin with dummy bf16 matmuls on a memset tile so the PE
    DVFS ramp (0.65 -> 1.2 -> 2.4 GHz over ~3us of busy time) is paid
    during the input-DMA wait instead of during real work.
  - The FFN prologue is DMA-transfer-bound (~2.7MB critical bytes). The
    first W1 chunk covers two h-tiles and is processed kd-OUTER across 6
    concurrent PSUM groups, so the PE consumes each arriving xt k-slice
    slower (1.3us) than the DMA delivers the next one (0.8us): once
    started, the PE never stalls and never re-pays the DVFS ramp.
    Prologue DMAs are issued on one queue in exact consumption order.
  - All FFN PSUM tiles share a single 8-bank ring (one tag).
  - Gating uses fp16 inputs (1 cycle/row vs fp32's 4, half the DMA bytes
    of fp32/fp32r): logit abs err ~9e-4 flips the top-2 set for 1 of 4096
    tokens on this input; end-to-end rel err 1.64e-2 < 2e-2 gate, verified
    bit-identical between device PE and the numpy model. Set GATE_DT to
    "f32" for exact fp32 gating (rel err 3.4e-3, ~8us slower) or "f32r".
  - The last mm2 d-tile runs its column chunks serially (72-wide chunk
    last) so the post-matmul drain is one small chunk, not three.
"""

import os
import sys
import types

import numpy as np
import ml_dtypes

import concourse.bass as bass
import concourse.mybir as mybir
import concourse.tile as tile
from concourse import bacc
from concourse.bass_utils import run_bass_kernel_spmd
from concourse.masks import make_identity

N_CORES = 8
P = 128
B, S, D, H, E = 2, 2048, 1024, 4096, 8
T = B * S
TG = T // N_CORES  # tokens per core for gating
BF16 = ml_dtypes.bfloat16
GATE_DT = "f16"  # "f16" | "f32r" | "f32"

AF = mybir.ActivationFunctionType
ALU = mybir.AluOpType
AX = mybir.AxisListType
F32 = mybir.dt.float32
F32R = mybir.dt.float32r
F16 = mybir.dt.float16
BF = mybir.dt.bfloat16


def _install_profile_hook():
    """Register the antenv.axon_hooks NTFF hook this image lacks, so
    BASS_TRACE=1 profiling works. Harmless no-op on failure."""
    try:
        if "antenv.axon_hooks" in sys.modules:
            return
        import antenv
        from trn_agent_boot.trn_boot import _ntff_profile_via_ctypes

        mod = types.ModuleType("antenv.axon_hooks")
        _h = [None]
        mod.set_axon_ntff_profile_hook = lambda h: _h.__setitem__(0, h)
        mod.get_axon_ntff_profile_hook = lambda: _h[0]
        sys.modules["antenv.axon_hooks"] = mod
        antenv.axon_hooks = mod
        so = "/opt/axon/libaxon_pjrt.so"
        if os.path.exists(so):
            mod.set_axon_ntff_profile_hook(_ntff_profile_via_ctypes(so))
    except Exception:
        pass


_install_profile_hook()

_NC_CACHE = {}


def _warmup(nc, cst, ps, tag, n_warm):
    """Dummy matmuls on a memset tile: ramp the PE clock while input DMAs
    are in flight. The psum scratch shares an existing pool tag's ring (it
    is never read; later real allocations just reuse the bank)."""
    warm_sb = cst.tile([P, 512], BF, name="warm_sb")
    nc.gpsimd.memset(warm_sb[:], 0)
    warm_ps = ps.tile([P, 512], F32, tag=tag, name="warm_ps")
    for _ in range(n_warm):
        nc.tensor.matmul(warm_ps[:], warm_sb[:, :P], warm_sb[:], start=True, stop=True)


def _build_gate_nc():
    """Launch A: per-core gating for TG tokens.

    Inputs : xtg [D, TG] (token slice, transposed), wg [D, E] — both in
             GATE_DT precision (fp32 bits for f32/f32r, fp16 for f16).
    Output : wout [TG, E] f32 — renormalized top-2 combine weights, dense
             over E (zero where expert not selected).
    """
    key = ("gate", TG, GATE_DT)
    if key in _NC_CACHE:
        return _NC_CACHE[key]
    gdt = {"f16": F16, "f32r": F32R, "f32": F32}[GATE_DT]
    ddt = F16 if GATE_DT == "f16" else F32
    nc = bacc.Bacc("TRN2", target_bir_lowering=False, debug=False, num_devices=N_CORES)
    xtg = nc.dram_tensor("xtg", [D, TG], ddt, kind="ExternalInput")
    wg = nc.dram_tensor("wg", [D, E], ddt, kind="ExternalInput")
    wout = nc.dram_tensor("wout", [TG, E], F32, kind="ExternalOutput")
    KD = D // P
    TT = TG // P
    HALF = TG // 2
    with tile.TileContext(nc) as tc:
        with (
            tc.tile_pool(name="cst", bufs=1) as cst,
            tc.tile_pool(name="wk", bufs=4) as wk,
            tc.tile_pool(name="ps", bufs=4, space="PSUM") as ps,
        ):
            _warmup(nc, cst, ps, "pg", 3)
            # All input DMAs on ONE queue in consumption order: each DMA is
            # ~16 descriptors processed round-robin across in-flight jobs,
            # so concurrent jobs delay the completion (sem>=16) of the
            # critical first ones.
            wg_sb = cst.tile([P, KD, E], gdt)
            wg_ap = wg.ap().rearrange("(kd p) e -> p kd e", p=P)
            # wg is tiny: its trigger runs on the scalar/Activation queue in
            # parallel with the first xtg group's on the sync queue.
            nc.scalar.dma_start(wg_sb[:], wg_ap.bitcast(gdt))
            ident = cst.tile([E, E], F32)
            make_identity(nc, ident[:])
            # xtg in 4 groups of [1, 2, 2, 3] k-slices: the first (small)
            # group completes earliest so matmuls start as soon as possible;
            # later (bigger) groups amortize the ~650ns per-DMA trigger.
            kgs = [1, 2, 2, 3]
            koff = [0, 1, 3, 5]
            xtg_sb = [
                cst.tile([P, kg, TG], gdt, name=f"xtg_{g}") for g, kg in enumerate(kgs)
            ]
            xtg_ap = xtg.ap().rearrange("(kd p) t -> p kd t", p=P).bitcast(gdt)
            for g, kg in enumerate(kgs):
                nc.sync.dma_start(xtg_sb[g][:], xtg_ap[:, koff[g] : koff[g] + kg, :])
            kd2g = [0, 1, 1, 2, 2, 3, 3, 3]
            wout_ap = wout.ap().rearrange("(tt p) e -> p tt e", p=P)
            # logits^T accumulated over k-tiles, split in two column halves
            # so top-k for the first half overlaps the second half's matmuls
            lt_sb = wk.tile([E, TG], F32, tag="lt")
            for hf in range(2):
                pl = ps.tile([E, HALF], F32, tag="pl", name=f"pl{hf}")
                for kd in range(KD):
                    nc.tensor.matmul(
                        pl[:],
                        wg_sb[:, kd, :],
                        xtg_sb[kd2g[kd]][:, kd - koff[kd2g[kd]], hf * HALF : (hf + 1) * HALF],
                        start=(kd == 0),
                        stop=(kd == KD - 1),
                    )
                nc.scalar.copy(lt_sb[:, hf * HALF : (hf + 1) * HALF], pl[:])
            for tt in range(TT):
                # transpose [E, 128] -> [128, E] so tokens sit on partitions
                pg = ps.tile([P, E], F32, tag="pg")
                nc.tensor.transpose(
                    pg[:], lt_sb[:, tt * P : (tt + 1) * P], ident[:]
                )
                logits = wk.tile([P, E], F32, tag="logits")
                nc.scalar.copy(logits[:], pg[:])
                top8 = wk.tile([P, 8], F32, tag="top8")
                nc.vector.max(out=top8[:], in_=logits[:])
                negm1 = wk.tile([P, 1], F32, tag="negm1")
                nc.vector.tensor_scalar_mul(negm1[:], top8[:, 0:1], -1.0)
                mask = wk.tile([P, E], F32, tag="mask")
                nc.vector.tensor_scalar(
                    out=mask[:],
                    in0=logits[:],
                    scalar1=top8[:, 1:2],
                    scalar2=None,
                    op0=ALU.is_ge,
                )
                ex = wk.tile([P, E], F32, tag="ex")
                nc.scalar.activation(ex[:], logits[:], AF.Exp, bias=negm1[:])
                wv = wk.tile([P, E], F32, tag="wv")
                nc.vector.tensor_mul(wv[:], ex[:], mask[:])
                ssum = wk.tile([P, 1], F32, tag="ssum")
                nc.vector.reduce_sum(ssum[:], wv[:], axis=AX.X)
                rec = wk.tile([P, 1], F32, tag="rec")
                nc.vector.reciprocal(rec[:], ssum[:])
                wn = wk.tile([P, E], F32, tag="wn")
                nc.vector.tensor_scalar_mul(wn[:], wv[:], rec[:])
                nc.gpsimd.dma_start(wout_ap[:, tt, :], wn[:])
    nc.compile()
    _NC_CACHE[key] = nc
    return nc


def _build_ffn_nc(C):
    """Launch B: per-core expert FFN over C (padded) routed tokens.

    Inputs : xt  [D, C]  bf16 — routed tokens, transposed
             w1 [D, H]  bf16, w2 [H, D] bf16 — this expert's weights
             b1r [P, H/P] f32, b2r [P, D/P] f32 — biases, partition-major
             wc [P, C] f32 — combine weights, replicated across partitions
    Output : yt [D, C] f32 — w * (gelu(x W1 + b1) W2 + b2), transposed
    """
    key = ("ffn", C)
    if key in _NC_CACHE:
        return _NC_CACHE[key]
    assert C % 8 == 0
    KD = D // P  # 8 k-tiles over D
    KH = H // P  # 32 k-tiles over H
    # First W1 chunk covers 2 h-tiles and is processed kd-outer (see module
    # docstring); later chunks stream 512 columns at a time.
    h_chunks = [256, 256] + [512] * 7
    assert sum(h_chunks) == H
    PRO_HT = h_chunks[0] // P  # h-tiles in the kd-outer prologue
    DC = 256  # d columns per W2 dma chunk
    n_off = list(range(0, C, 512))
    n_szs = [min(512, C - o) for o in n_off]
    NCH = len(n_off)

    nc = bacc.Bacc("TRN2", target_bir_lowering=False, debug=False, num_devices=N_CORES)
    xt = nc.dram_tensor("xt", [D, C], BF, kind="ExternalInput")
    w1 = nc.dram_tensor("w1", [D, H], BF, kind="ExternalInput")
    w2 = nc.dram_tensor("w2", [H, D], BF, kind="ExternalInput")
    b1r = nc.dram_tensor("b1r", [P, H // P], F32, kind="ExternalInput")
    b2r = nc.dram_tensor("b2r", [P, D // P], F32, kind="ExternalInput")
    wc = nc.dram_tensor("wc", [P, C], F32, kind="ExternalInput")
    yt = nc.dram_tensor("yt", [D, C], F32, kind="ExternalOutput")

    with tile.TileContext(nc) as tc:
        with (
            tc.tile_pool(name="cst", bufs=1) as cst,
            tc.tile_pool(name="w1p", bufs=3) as w1p,
            tc.tile_pool(name="w2p", bufs=2) as w2p,
            tc.tile_pool(name="outp", bufs=6) as outp,
            tc.tile_pool(name="ps", bufs=8, space="PSUM") as ps,
        ):
            _warmup(nc, cst, ps, "ps", 12)
            # Prologue DMAs all on the sync queue, in exact consumption
            # order: first W1 chunk, then the xt k-slices.
            w1_c0 = w1p.tile([P, KD, 512], BF, tag="w1c", name="w1_c0")
            nc.sync.dma_start(
                w1_c0[:, :, : h_chunks[0]],
                w1.ap()[:, 0 : h_chunks[0]].rearrange("(kd p) h -> p kd h", p=P),
            )
            # one tile per k-slice: per-slice DMA completion unblocks the
            # prologue matmuls as each slice lands (whole-tile dependency)
            xt_sb = [cst.tile([P, C], BF, name=f"xt_k{kd}") for kd in range(KD)]
            xt_ap = xt.ap().rearrange("(kd p) c -> p kd c", p=P)
            for kd in range(KD):
                nc.sync.dma_start(xt_sb[kd][:], xt_ap[:, kd, :])
            # Second W1 chunk right after xt: needed at h-tile 2 (~23us).
            w1_c1 = w1p.tile([P, KD, 512], BF, tag="w1c", name="w1_c1")
            nc.sync.dma_start(
                w1_c1[:, :, : h_chunks[1]],
                w1.ap()[:, h_chunks[0] : h_chunks[0] + h_chunks[1]].rearrange(
                    "(kd p) h -> p kd h", p=P
                ),
            )
            # Latency-tolerant loads go on the SAME sync queue AFTER the
            # critical prologue jobs: each DMA is ~16 descriptors processed
            # round-robin across in-flight jobs, so a concurrent gpsimd-queue
            # job would delay the completion (sem>=16) of w1_c0/xt_k0.
            b1_sb = cst.tile([P, H // P], F32)
            nc.sync.dma_start(b1_sb[:], b1r.ap())
            wc_sb = cst.tile([P, C], F32)
            nc.sync.dma_start(wc_sb[:], wc.ap())
            b2_sb = cst.tile([P, D // P], F32)
            nc.sync.dma_start(b2_sb[:], b2r.ap())
            ht_sb = cst.tile([P, KH, C], BF)

            # ---- mm1: ht[h, c] = gelu(sum_d w1[d, h] * xt[d, c] + b1[h]) ----
            # prologue: kd-outer over PRO_HT h-tiles x NCH chunks
            pro_ps = [
                ps.tile([P, 512], F32, tag="ps", name=f"pro_{ht}_{n}")
                for ht in range(PRO_HT)
                for n in range(NCH)
            ]
            for kd in range(KD):
                for ht in range(PRO_HT):
                    for n in range(NCH):
                        nc.tensor.matmul(
                            pro_ps[ht * NCH + n][:, : n_szs[n]],
                            w1_c0[:, kd, ht * P : (ht + 1) * P],
                            xt_sb[kd][:, n_off[n] : n_off[n] + n_szs[n]],
                            start=(kd == 0),
                            stop=(kd == KD - 1),
                        )
            for ht in range(PRO_HT):
                for n in range(NCH):
                    nc.scalar.activation(
                        ht_sb[:, ht, n_off[n] : n_off[n] + n_szs[n]],
                        pro_ps[ht * NCH + n][:, : n_szs[n]],
                        AF.Gelu,
                        bias=b1_sb[:, ht : ht + 1],
                    )
            # steady state: h-outer, kd-inner
            h_off = h_chunks[0]
            h_tile = PRO_HT
            for hc, hsz in enumerate(h_chunks[1:], start=1):
                if hc == 1:
                    w1_c = w1_c1  # DMA already issued in the prologue
                else:
                    w1_c = w1p.tile([P, KD, 512], BF, tag="w1c", name=f"w1_c{hc}")
                    nc.sync.dma_start(
                        w1_c[:, :, :hsz],
                        w1.ap()[:, h_off : h_off + hsz].rearrange(
                            "(kd p) h -> p kd h", p=P
                        ),
                    )
                for hs in range(hsz // P):
                    psum_ts = [ps.tile([P, 512], F32, tag="ps", name=f"ps1_{h_tile}_{n}") for n in range(NCH)]
                    for kd in range(KD):
                        for n in range(NCH):
                            nc.tensor.matmul(
                                psum_ts[n][:, : n_szs[n]],
                                w1_c[:, kd, hs * P : (hs + 1) * P],
                                xt_sb[kd][:, n_off[n] : n_off[n] + n_szs[n]],
                                start=(kd == 0),
                                stop=(kd == KD - 1),
                            )
                    for n in range(NCH):
                        nc.scalar.activation(
                            ht_sb[:, h_tile, n_off[n] : n_off[n] + n_szs[n]],
                            psum_ts[n][:, : n_szs[n]],
                            AF.Gelu,
                            bias=b1_sb[:, h_tile : h_tile + 1],
                        )
                    h_tile += 1
                h_off += hsz

            # ---- mm2: yt[d, c] = (sum_h w2[h, d] * ht[h, c] + b2[d]) * wc[c] ----
            yt_ap = yt.ap().rearrange("(dt p) c -> p dt c", p=P)

            def mm2_out(d_tile, n, psum_t):
                nsz = n_szs[n]
                tmp = outp.tile([P, 512], F32, tag="tmp")
                nc.scalar.activation(
                    tmp[:, :nsz],
                    psum_t[:, :nsz],
                    AF.Identity,
                    bias=b2_sb[:, d_tile : d_tile + 1],
                )
                out_t = outp.tile([P, 512], F32, tag="out")
                nc.vector.tensor_mul(
                    out_t[:, :nsz],
                    tmp[:, :nsz],
                    wc_sb[:, n_off[n] : n_off[n] + nsz],
                )
                nc.sync.dma_start(
                    yt_ap[:, d_tile, n_off[n] : n_off[n] + nsz],
                    out_t[:, :nsz],
                )

            n_dt = D // P
            for dc in range(D // DC):
                w2_c = w2p.tile([P, KH, DC], BF, tag="w2c")
                nc.sync.dma_start(
                    w2_c[:],
                    w2.ap()[:, dc * DC : (dc + 1) * DC].rearrange(
                        "(kh p) d -> p kh d", p=P
                    ),
                )
                for dsx in range(DC // P):
                    d_tile = dc * (DC // P) + dsx
                    psum_ts = [ps.tile([P, 512], F32, tag="ps", name=f"ps2_{d_tile}_{n}") for n in range(NCH)]
                    if d_tile < n_dt - 1:
                        # chunk-interleaved: steady-state cadence
                        for kh in range(KH):
                            for n in range(NCH):
                                nc.tensor.matmul(
                                    psum_ts[n][:, : n_szs[n]],
                                    w2_c[:, kh, dsx * P : (dsx + 1) * P],
                                    ht_sb[:, kh, n_off[n] : n_off[n] + n_szs[n]],
                                    start=(kh == 0),
                                    stop=(kh == KH - 1),
                                )
                        for n in range(NCH):
                            mm2_out(d_tile, n, psum_ts[n])
                    else:
                        # final d-tile: chunk-serial so the tail drain after
                        # the very last matmul is a single (small) chunk
                        for n in range(NCH):
                            for kh in range(KH):
                                nc.tensor.matmul(
                                    psum_ts[n][:, : n_szs[n]],
                                    w2_c[:, kh, dsx * P : (dsx + 1) * P],
                                    ht_sb[:, kh, n_off[n] : n_off[n] + n_szs[n]],
                                    start=(kh == 0),
                                    stop=(kh == KH - 1),
                                )
                            mm2_out(d_tile, n, psum_ts[n])
    nc.compile()
    _NC_CACHE[key] = nc
    return nc


# results of the most recent kernel() call, for test harness introspection
last_results = {}


def kernel(**inputs):
    x = np.asarray(inputs["x"], np.float32)
    Wg = np.asarray(inputs["Wg"], np.float32)
    W1 = np.asarray(inputs["W1"], np.float32)
    b1 = np.asarray(inputs["b1"], np.float32)
    W2 = np.asarray(inputs["W2"], np.float32)
    b2 = np.asarray(inputs["b2"], np.float32)
    assert x.shape == (B, S, D) and Wg.shape == (D, E)
    assert W1.shape == (E, D, H) and W2.shape == (E, H, D)

    xf = np.ascontiguousarray(x.reshape(T, D))
    core_ids = list(range(N_CORES))

    # ---- Launch A: gating on device (data-parallel over tokens) ----
    ncA = _build_gate_nc()
    gcast = np.float16 if GATE_DT == "f16" else np.float32
    in_maps_a = [
        {
            "xtg": np.ascontiguousarray(xf[m * TG : (m + 1) * TG].T).astype(gcast),
            "wg": Wg.astype(gcast),
        }
        for m in range(N_CORES)
    ]
    resA = run_bass_kernel_spmd(ncA, in_maps_a, core_ids=core_ids)
    w_full = np.concatenate([resA.results[m]["wout"] for m in range(N_CORES)], axis=0)

    # ---- Host routing: build per-expert token lists from device weights ----
    idx_list, wval_list = [], []
    max_cnt = 1
    for e in range(E):
        idx = np.nonzero(w_full[:, e] > 0.0)[0]
        idx_list.append(idx)
        wval_list.append(w_full[idx, e].astype(np.float32))
        max_cnt = max(max_cnt, len(idx))
    C = ((max_cnt + 7) // 8) * 8

    # ---- Launch B: expert-parallel FFN ----
    ncB = _build_ffn_nc(C)
    in_maps_b = []
    for e in range(E):
        idx = idx_list[e]
        cnt = len(idx)
        xt = np.zeros((D, C), BF16)
        xt[:, :cnt] = xf[idx].T.astype(BF16)
        wcv = np.zeros((C,), np.float32)
        wcv[:cnt] = wval_list[e]
        in_maps_b.append(
            {
                "xt": xt,
                "w1": np.ascontiguousarray(W1[e].astype(BF16)),
                "w2": np.ascontiguousarray(W2[e].astype(BF16)),
                "b1r": np.ascontiguousarray(b1[e].reshape(H // P, P).T),
                "b2r": np.ascontiguousarray(b2[e].reshape(D // P, P).T),
                "wc": np.ascontiguousarray(np.broadcast_to(wcv, (P, C))),
            }
        )
    resB = run_bass_kernel_spmd(ncB, in_maps_b, core_ids=core_ids)

    # ---- Host unshard: scatter-add weighted partial outputs ----
    out = np.zeros((T, D), np.float32)
    for e in range(E):
        idx = idx_list[e]
        cnt = len(idx)
        if cnt:
            out[idx] += resB.results[e]["yt"][:, :cnt].T

    last_results["gate"] = resA
    last_results["ffn"] = resB
    return out.reshape(B, S, D)


# revision 25
# speedup vs baseline: 1.0053x; 1.0053x over previous
"""Mixture-of-Experts (top-2 of 8) Trainium2 kernel, expert-parallel over 8 NeuronCores.

Strategy (per the expert-parallel sharding hint):
  Launch A (data-parallel gating): each core computes gating logits for T/8
    tokens (x_slice @ Wg on the PE in fp16, 1 cycle/row), then top-2
    selection + renormalized combine weights with vector/scalar ops.
    Output: dense [T, E] combine weights (zero for unselected experts).
  Host routing ("all-to-all dispatch"): from the device-computed combine
    weights, build per-expert token index lists, gather+transpose+bf16-cast
    the routed tokens for each expert, pad to a common capacity C.
  Launch B (expert-parallel FFN): core e holds expert e's weights. Computes
    h^T = gelu(W1^T x^T + b1), y^T = (W2^T h^T + b2) * w on the PE in bf16
    with fp32 accumulation; biases added exactly in fp32 on the scalar
    engine; combine weight applied on the vector engine.
  Host unshard: scatter-add the 8 weighted partial outputs into [T, D].

Perf notes (measured on HW):
  - Both launches begin with dummy bf16 matmuls on a memset tile so the PE
    DVFS ramp (0.65 -> 1.2 -> 2.4 GHz over ~3us of busy time) is paid
    during the input-DMA wait instead of during real work.
  - The FFN prologue is DMA-transfer-bound (~2.7MB critical bytes). The
    first W1 chunk covers two h-tiles and is processed kd-OUTER across 6
    concurrent PSUM groups, so the PE consumes each arriving xt k-slice
    slower (1.3us) than the DMA delivers the next one (0.8us): once
    started, the PE never stalls and never re-pays the DVFS ramp.
    Prologue DMAs are issued on one queue in exact consumption order.
  - All FFN PSUM tiles share a single 8-bank ring (one tag).
  - Gating uses fp16 inputs (1 cycle/row vs fp32's 4, half the DMA bytes
    of fp32/fp32r): logit abs err ~9e-4 flips the top-2 set for 1 of 4096
    tokens on this input; end-to-end rel err 1.64e-2 < 2e-2 gate, verified
    bit-identical between device PE and the numpy model. Set GATE_DT to
    "f32" for exact fp32 gating (rel err 3.4e-3, ~8us slower) or "f32r".
  - The last mm2 d-tile runs its column chunks serially (72-wide chunk
    last) so the post-matmul drain is one small chunk, not three.
"""

import os
import sys
import types

import numpy as np
import ml_dtypes

import concourse.bass as bass
import concourse.mybir as mybir
import concourse.tile as tile
from concourse import bacc
from concourse.bass_utils import run_bass_kernel_spmd
from concourse.masks import make_identity

N_CORES = 8
P = 128
B, S, D, H, E = 2, 2048, 1024, 4096, 8
T = B * S
TG = T // N_CORES  # tokens per core for gating
BF16 = ml_dtypes.bfloat16
GATE_DT = "f16"  # "f16" | "f32r" | "f32"

AF = mybir.ActivationFunctionType
ALU = mybir.AluOpType
AX = mybir.AxisListType
F32 = mybir.dt.float32
F32R = mybir.dt.float32r
F16 = mybir.dt.float16
BF = mybir.dt.bfloat16


def _install_profile_hook():
    """Register the antenv.axon_hooks NTFF hook this image lacks, so
    BASS_TRACE=1 profiling works. Harmless no-op on failure."""
    try:
        if "antenv.axon_hooks" in sys.modules:
            return
        import antenv
        from trn_agent_boot.trn_boot import _ntff_profile_via_ctypes

        mod = types.ModuleType("antenv.axon_hooks")
        _h = [None]
        mod.set_axon_ntff_profile_hook = lambda h: _h.__setitem__(0, h)
        mod.get_axon_ntff_profile_hook = lambda: _h[0]
        sys.modules["antenv.axon_hooks"] = mod
        antenv.axon_hooks = mod
        so = "/opt/axon/libaxon_pjrt.so"
        if os.path.exists(so):
            mod.set_axon_ntff_profile_hook(_ntff_profile_via_ctypes(so))
    except Exception:
        pass


_install_profile_hook()

_NC_CACHE = {}


def _warmup(nc, cst, ps, tag, n_warm):
    """Dummy matmuls on a memset tile: ramp the PE clock while input DMAs
    are in flight. The psum scratch shares an existing pool tag's ring (it
    is never read; later real allocations just reuse the bank)."""
    warm_sb = cst.tile([P, 512], BF, name="warm_sb")
    nc.gpsimd.memset(warm_sb[:], 0)
    warm_ps = ps.tile([P, 512], F32, tag=tag, name="warm_ps")
    for _ in range(n_warm):
        nc.tensor.matmul(warm_ps[:], warm_sb[:, :P], warm_sb[:], start=True, stop=True)


def _build_gate_nc():
    """Launch A: per-core gating for TG tokens.

    Inputs : xtg [D, TG] (token slice, transposed), wg [D, E] — both in
             GATE_DT precision (fp32 bits for f32/f32r, fp16 for f16).
    Output : wout [TG, E] f32 — renormalized top-2 combine weights, dense
             over E (zero where expert not selected).
    """
    key = ("gate", TG, GATE_DT)
    if key in _NC_CACHE:
        return _NC_CACHE[key]
    gdt = {"f16": F16, "f32r": F32R, "f32": F32}[GATE_DT]
    ddt = F16 if GATE_DT == "f16" else F32
    nc = bacc.Bacc("TRN2", target_bir_lowering=False, debug=False, num_devices=N_CORES)
    xtg = nc.dram_tensor("xtg", [D, TG], ddt, kind="ExternalInput")
    wg = nc.dram_tensor("wg", [D, E], ddt, kind="ExternalInput")
    wout = nc.dram_tensor("wout", [TG, E], F32, kind="ExternalOutput")
    KD = D // P
    TT = TG // P
    HALF = TG // 2
    with tile.TileContext(nc) as tc:
        with (
            tc.tile_pool(name="cst", bufs=1) as cst,
            tc.tile_pool(name="wk", bufs=4) as wk,
            tc.tile_pool(name="ps", bufs=4, space="PSUM") as ps,
        ):
            _warmup(nc, cst, ps, "pg", 3)
            # All input DMAs on ONE queue in consumption order: each DMA is
            # ~16 descriptors processed round-robin across in-flight jobs,
            # so concurrent jobs delay the completion (sem>=16) of the
            # critical first ones.
            wg_sb = cst.tile([P, KD, E], gdt)
            wg_ap = wg.ap().rearrange("(kd p) e -> p kd e", p=P)
            # wg is tiny: its trigger runs on the scalar/Activation queue in
            # parallel with the first xtg group's on the sync queue.
            nc.scalar.dma_start(wg_sb[:], wg_ap.bitcast(gdt))
            ident = cst.tile([E, E], F32)
            make_identity(nc, ident[:])
            # xtg in 4 groups of [1, 2, 2, 3] k-slices: the first (small)
            # group completes earliest so matmuls start as soon as possible;
            # later (bigger) groups amortize the ~650ns per-DMA trigger.
            kgs = [1, 2, 2, 3]
            koff = [0, 1, 3, 5]
            xtg_sb = [
                cst.tile([P, kg, TG], gdt, name=f"xtg_{g}") for g, kg in enumerate(kgs)
            ]
            xtg_ap = xtg.ap().rearrange("(kd p) t -> p kd t", p=P).bitcast(gdt)
            for g, kg in enumerate(kgs):
                nc.sync.dma_start(xtg_sb[g][:], xtg_ap[:, koff[g] : koff[g] + kg, :])
            kd2g = [0, 1, 1, 2, 2, 3, 3, 3]
            wout_ap = wout.ap().rearrange("(tt p) e -> p tt e", p=P)
            # logits^T accumulated over k-tiles, split in two column halves
            # so top-k for the first half overlaps the second half's matmuls
            lt_sb = wk.tile([E, TG], F32, tag="lt")
            for hf in range(2):
                pl = ps.tile([E, HALF], F32, tag="pl", name=f"pl{hf}")
                for kd in range(KD):
                    nc.tensor.matmul(
                        pl[:],
                        wg_sb[:, kd, :],
                        xtg_sb[kd2g[kd]][:, kd - koff[kd2g[kd]], hf * HALF : (hf + 1) * HALF],
                        start=(kd == 0),
                        stop=(kd == KD - 1),
                    )
                nc.scalar.copy(lt_sb[:, hf * HALF : (hf + 1) * HALF], pl[:])
            for tt in range(TT):
                # transpose [E, 128] -> [128, E] so tokens sit on partitions
                pg = ps.tile([P, E], F32, tag="pg")
                nc.tensor.transpose(
                    pg[:], lt_sb[:, tt * P : (tt + 1) * P], ident[:]
                )
                logits = wk.tile([P, E], F32, tag="logits")
                nc.scalar.copy(logits[:], pg[:])
                top8 = wk.tile([P, 8], F32, tag="top8")
                nc.vector.max(out=top8[:], in_=logits[:])
                negm1 = wk.tile([P, 1], F32, tag="negm1")
                nc.vector.tensor_scalar_mul(negm1[:], top8[:, 0:1], -1.0)
                mask = wk.tile([P, E], F32, tag="mask")
                nc.vector.tensor_scalar(
                    out=mask[:],
                    in0=logits[:],
                    scalar1=top8[:, 1:2],
                    scalar2=None,
                    op0=ALU.is_ge,
                )
                ex = wk.tile([P, E], F32, tag="ex")
                nc.scalar.activation(ex[:], logits[:], AF.Exp, bias=negm1[:])
                wv = wk.tile([P, E], F32, tag="wv")
                nc.vector.tensor_mul(wv[:], ex[:], mask[:])
                ssum = wk.tile([P, 1], F32, tag="ssum")
                nc.vector.reduce_sum(ssum[:], wv[:], axis=AX.X)
                rec = wk.tile([P, 1], F32, tag="rec")
                nc.vector.reciprocal(rec[:], ssum[:])
                wn = wk.tile([P, E], F32, tag="wn")
                nc.vector.tensor_scalar_mul(wn[:], wv[:], rec[:])
                nc.gpsimd.dma_start(wout_ap[:, tt, :], wn[:])
    nc.compile()
    _NC_CACHE[key] = nc
    return nc


def _build_ffn_nc(C):
    """Launch B: per-core expert FFN over C (padded) routed tokens.

    Inputs : xt  [D, C]  bf16 — routed tokens, transposed
             w1 [D, H]  bf16, w2 [H, D] bf16 — this expert's weights
             b1r [P, H/P] f32, b2r [P, D/P] f32 — biases, partition-major
             wc [P, C] f32 — combine weights, replicated across partitions
    Output : yt [D, C] f32 — w * (gelu(x W1 + b1) W2 + b2), transposed
    """
    key = ("ffn", C)
    if key in _NC_CACHE:
        return _NC_CACHE[key]
    assert C % 4 == 0
    KD = D // P  # 8 k-tiles over D
    KH = H // P  # 32 k-tiles over H
    # First W1 chunk covers 2 h-tiles and is processed kd-outer (see module
    # docstring); later chunks stream 512 columns at a time.
    h_chunks = [256, 256] + [512] * 7
    assert sum(h_chunks) == H
    PRO_HT = h_chunks[0] // P  # h-tiles in the kd-outer prologue
    DC = 256  # d columns per W2 dma chunk
    n_off = list(range(0, C, 512))
    n_szs = [min(512, C - o) for o in n_off]
    NCH = len(n_off)

    nc = bacc.Bacc("TRN2", target_bir_lowering=False, debug=False, num_devices=N_CORES)
    xt = nc.dram_tensor("xt", [D, C], BF, kind="ExternalInput")
    w1 = nc.dram_tensor("w1", [D, H], BF, kind="ExternalInput")
    w2 = nc.dram_tensor("w2", [H, D], BF, kind="ExternalInput")
    b1r = nc.dram_tensor("b1r", [P, H // P], F32, kind="ExternalInput")
    b2r = nc.dram_tensor("b2r", [P, D // P], F32, kind="ExternalInput")
    wc = nc.dram_tensor("wc", [P, C], F32, kind="ExternalInput")
    yt = nc.dram_tensor("yt", [D, C], F32, kind="ExternalOutput")

    with tile.TileContext(nc) as tc:
        with (
            tc.tile_pool(name="cst", bufs=1) as cst,
            tc.tile_pool(name="w1p", bufs=3) as w1p,
            tc.tile_pool(name="w2p", bufs=2) as w2p,
            tc.tile_pool(name="outp", bufs=6) as outp,
            tc.tile_pool(name="ps", bufs=8, space="PSUM") as ps,
        ):
            _warmup(nc, cst, ps, "ps", 12)
            # Prologue DMAs all on the sync queue, in exact consumption
            # order: first W1 chunk, then the xt k-slices.
            w1_c0 = w1p.tile([P, KD, 512], BF, tag="w1c", name="w1_c0")
            nc.sync.dma_start(
                w1_c0[:, :, : h_chunks[0]],
                w1.ap()[:, 0 : h_chunks[0]].rearrange("(kd p) h -> p kd h", p=P),
            )
            # one tile per k-slice: per-slice DMA completion unblocks the
            # prologue matmuls as each slice lands (whole-tile dependency)
            xt_sb = [cst.tile([P, C], BF, name=f"xt_k{kd}") for kd in range(KD)]
            xt_ap = xt.ap().rearrange("(kd p) c -> p kd c", p=P)
            for kd in range(KD):
                nc.sync.dma_start(xt_sb[kd][:], xt_ap[:, kd, :])
            # Second W1 chunk right after xt: needed at h-tile 2 (~23us).
            w1_c1 = w1p.tile([P, KD, 512], BF, tag="w1c", name="w1_c1")
            nc.sync.dma_start(
                w1_c1[:, :, : h_chunks[1]],
                w1.ap()[:, h_chunks[0] : h_chunks[0] + h_chunks[1]].rearrange(
                    "(kd p) h -> p kd h", p=P
                ),
            )
            # Latency-tolerant loads go on the SAME sync queue AFTER the
            # critical prologue jobs: each DMA is ~16 descriptors processed
            # round-robin across in-flight jobs, so a concurrent gpsimd-queue
            # job would delay the completion (sem>=16) of w1_c0/xt_k0.
            b1_sb = cst.tile([P, H // P], F32)
            nc.sync.dma_start(b1_sb[:], b1r.ap())
            wc_sb = cst.tile([P, C], F32)
            nc.sync.dma_start(wc_sb[:], wc.ap())
            b2_sb = cst.tile([P, D // P], F32)
            nc.sync.dma_start(b2_sb[:], b2r.ap())
            ht_sb = cst.tile([P, KH, C], BF)

            # ---- mm1: ht[h, c] = gelu(sum_d w1[d, h] * xt[d, c] + b1[h]) ----
            # prologue: kd-outer over PRO_HT h-tiles x NCH chunks
            pro_ps = [
                ps.tile([P, 512], F32, tag="ps", name=f"pro_{ht}_{n}")
                for ht in range(PRO_HT)
                for n in range(NCH)
            ]
            for kd in range(KD):
                for ht in range(PRO_HT):
                    for n in range(NCH):
                        nc.tensor.matmul(
                            pro_ps[ht * NCH + n][:, : n_szs[n]],
                            w1_c0[:, kd, ht * P : (ht + 1) * P],
                            xt_sb[kd][:, n_off[n] : n_off[n] + n_szs[n]],
                            start=(kd == 0),
                            stop=(kd == KD - 1),
                        )
            for ht in range(PRO_HT):
                for n in range(NCH):
                    nc.scalar.activation(
                        ht_sb[:, ht, n_off[n] : n_off[n] + n_szs[n]],
                        pro_ps[ht * NCH + n][:, : n_szs[n]],
                        AF.Gelu,
                        bias=b1_sb[:, ht : ht + 1],
                    )
            # steady state: h-outer, kd-inner
            h_off = h_chunks[0]
            h_tile = PRO_HT
            for hc, hsz in enumerate(h_chunks[1:], start=1):
                if hc == 1:
                    w1_c = w1_c1  # DMA already issued in the prologue
                else:
                    w1_c = w1p.tile([P, KD, 512], BF, tag="w1c", name=f"w1_c{hc}")
                    nc.sync.dma_start(
                        w1_c[:, :, :hsz],
                        w1.ap()[:, h_off : h_off + hsz].rearrange(
                            "(kd p) h -> p kd h", p=P
                        ),
                    )
                for hs in range(hsz // P):
                    psum_ts = [ps.tile([P, 512], F32, tag="ps", name=f"ps1_{h_tile}_{n}") for n in range(NCH)]
                    for kd in range(KD):
                        for n in range(NCH):
                            nc.tensor.matmul(
                                psum_ts[n][:, : n_szs[n]],
                                w1_c[:, kd, hs * P : (hs + 1) * P],
                                xt_sb[kd][:, n_off[n] : n_off[n] + n_szs[n]],
                                start=(kd == 0),
                                stop=(kd == KD - 1),
                            )
                    for n in range(NCH):
                        nc.scalar.activation(
                            ht_sb[:, h_tile, n_off[n] : n_off[n] + n_szs[n]],
                            psum_ts[n][:, : n_szs[n]],
                            AF.Gelu,
                            bias=b1_sb[:, h_tile : h_tile + 1],
                        )
                    h_tile += 1
                h_off += hsz

            # ---- mm2: yt[d, c] = (sum_h w2[h, d] * ht[h, c] + b2[d]) * wc[c] ----
            yt_ap = yt.ap().rearrange("(dt p) c -> p dt c", p=P)

            def mm2_out(d_tile, n, psum_t):
                nsz = n_szs[n]
                tmp = outp.tile([P, 512], F32, tag="tmp")
                nc.scalar.activation(
                    tmp[:, :nsz],
                    psum_t[:, :nsz],
                    AF.Identity,
                    bias=b2_sb[:, d_tile : d_tile + 1],
                )
                out_t = outp.tile([P, 512], F32, tag="out")
                nc.vector.tensor_mul(
                    out_t[:, :nsz],
                    tmp[:, :nsz],
                    wc_sb[:, n_off[n] : n_off[n] + nsz],
                )
                nc.sync.dma_start(
                    yt_ap[:, d_tile, n_off[n] : n_off[n] + nsz],
                    out_t[:, :nsz],
                )

            n_dt = D // P
            for dc in range(D // DC):
                w2_c = w2p.tile([P, KH, DC], BF, tag="w2c")
                nc.sync.dma_start(
                    w2_c[:],
                    w2.ap()[:, dc * DC : (dc + 1) * DC].rearrange(
                        "(kh p) d -> p kh d", p=P
                    ),
                )
                for dsx in range(DC // P):
                    d_tile = dc * (DC // P) + dsx
                    psum_ts = [ps.tile([P, 512], F32, tag="ps", name=f"ps2_{d_tile}_{n}") for n in range(NCH)]
                    if d_tile < n_dt - 1:
                        # chunk-interleaved: steady-state cadence
                        for kh in range(KH):
                            for n in range(NCH):
                                nc.tensor.matmul(
                                    psum_ts[n][:, : n_szs[n]],
                                    w2_c[:, kh, dsx * P : (dsx + 1) * P],
                                    ht_sb[:, kh, n_off[n] : n_off[n] + n_szs[n]],
                                    start=(kh == 0),
                                    stop=(kh == KH - 1),
                                )
                        for n in range(NCH):
                            mm2_out(d_tile, n, psum_ts[n])
                    else:
                        # final d-tile: chunk-serial so the tail drain after
                        # the very last matmul is a single (small) chunk
                        for n in range(NCH):
                            for kh in range(KH):
                                nc.tensor.matmul(
                                    psum_ts[n][:, : n_szs[n]],
                                    w2_c[:, kh, dsx * P : (dsx + 1) * P],
                                    ht_sb[:, kh, n_off[n] : n_off[n] + n_szs[n]],
                                    start=(kh == 0),
                                    stop=(kh == KH - 1),
                                )
                            mm2_out(d_tile, n, psum_ts[n])
    nc.compile()
    _NC_CACHE[key] = nc
    return nc


# results of the most recent kernel() call, for test harness introspection
last_results = {}


def kernel(**inputs):
    x = np.asarray(inputs["x"], np.float32)
    Wg = np.asarray(inputs["Wg"], np.float32)
    W1 = np.asarray(inputs["W1"], np.float32)
    b1 = np.asarray(inputs["b1"], np.float32)
    W2 = np.asarray(inputs["W2"], np.float32)
    b2 = np.asarray(inputs["b2"], np.float32)
    assert x.shape == (B, S, D) and Wg.shape == (D, E)
    assert W1.shape == (E, D, H) and W2.shape == (E, H, D)

    xf = np.ascontiguousarray(x.reshape(T, D))
    core_ids = list(range(N_CORES))

    # ---- Launch A: gating on device (data-parallel over tokens) ----
    ncA = _build_gate_nc()
    gcast = np.float16 if GATE_DT == "f16" else np.float32
    in_maps_a = [
        {
            "xtg": np.ascontiguousarray(xf[m * TG : (m + 1) * TG].T).astype(gcast),
            "wg": Wg.astype(gcast),
        }
        for m in range(N_CORES)
    ]
    resA = run_bass_kernel_spmd(ncA, in_maps_a, core_ids=core_ids)
    w_full = np.concatenate([resA.results[m]["wout"] for m in range(N_CORES)], axis=0)

    # ---- Host routing: build per-expert token lists from device weights ----
    idx_list, wval_list = [], []
    max_cnt = 1
    for e in range(E):
        idx = np.nonzero(w_full[:, e] > 0.0)[0]
        idx_list.append(idx)
        wval_list.append(w_full[idx, e].astype(np.float32))
        max_cnt = max(max_cnt, len(idx))
    # pad capacity only to 4 columns (f32 rows stay 16B-aligned): every
    # dead column costs 512 PE rows (~0.2us), so pad as little as possible
    C = ((max_cnt + 3) // 4) * 4

    # ---- Launch B: expert-parallel FFN ----
    ncB = _build_ffn_nc(C)
    in_maps_b = []
    for e in range(E):
        idx = idx_list[e]
        cnt = len(idx)
        xt = np.zeros((D, C), BF16)
        xt[:, :cnt] = xf[idx].T.astype(BF16)
        wcv = np.zeros((C,), np.float32)
        wcv[:cnt] = wval_list[e]
        in_maps_b.append(
            {
                "xt": xt,
                "w1": np.ascontiguousarray(W1[e].astype(BF16)),
                "w2": np.ascontiguousarray(W2[e].astype(BF16)),
                "b1r": np.ascontiguousarray(b1[e].reshape(H // P, P).T),
                "b2r": np.ascontiguousarray(b2[e].reshape(D // P, P).T),
                "wc": np.ascontiguousarray(np.broadcast_to(wcv, (P, C))),
            }
        )
    resB = run_bass_kernel_spmd(ncB, in_maps_b, core_ids=core_ids)

    # ---- Host unshard: scatter-add weighted partial outputs ----
    out = np.zeros((T, D), np.float32)
    for e in range(E):
        idx = idx_list[e]
        cnt = len(idx)
        if cnt:
            out[idx] += resB.results[e]["yt"][:, :cnt].T

    last_results["gate"] = resA
    last_results["ffn"] = resB
    return out.reshape(B, S, D)
